# revision 3
# baseline (speedup 1.0000x reference)
"""Trainium2 Bass kernel for nn_SparseEncoder (sparse autoencoder / top-k masking).

reference:
    pre   = act @ W_enc.T + b          # [4096 tokens, 16384 concepts]
    top32 = top_k(pre, 32) per token
    sparse= scatter(top32)             # zeros elsewhere
    out   = sparse @ W_emb.T           # [4096, 1024]

Sharding: data-parallel over tokens, 512 tokens per core on 8 cores, weights
replicated. Per core:
  Phase 1 (encode): stream W_enc^T tiles, fp32 PE matmuls accumulate pre_act
    [128t x 512c] tiles in PSUM (bias added via two K=1 fp16 rank-1 matmuls,
    exact to ~1e-7); evict to SBUF; DVE max8 per 256-concept chunk collects
    top-8 candidates (512/token); PE-transpose tiles and spill pre^T [c, t]
    to a DRAM scratch.
  Phase 1.5 (top-k): 4x (max8 + match_replace) over the 512 candidates gives
    the top-32 values; reduce_min -> per-token threshold; PE transpose +
    rank-1 ones matmul broadcasts thresholds to a [128, 512t] tile.
  Phase 2 (decode): read pre^T chunks back, mask (pre >= thr) * pre -> fp16
    sparse codes, fp16 PE matmuls accumulate out [tokens, 1024] over all
    16384 concepts in 8 PSUM banks.

fp32 encode is mandatory: top-32/33 gaps go down to 8.6e-6 on this input, so
the ~1e-4..1e-2 error of fp32r/bf16/fp16 matmuls would flip selections and
blow up the output error; measured PE fp32 error is ~7e-7. The decode only
needs value accuracy, so fp16 (~1e-3 on pre-scale values, ~1e-4 on outputs)
is fine there.
"""

import numpy as np

import concourse.bass as bass
import concourse.mybir as mybir
from concourse import bacc
from concourse.masks import make_identity
from concourse.tile import TileContext
from concourse.bass_utils import run_bass_kernel_spmd

FP32 = mybir.dt.float32
FP16 = mybir.dt.float16

B, S, D, C, K_TOP = 2, 2048, 1024, 16384, 32
N_CORES = 8
T = (B * S) // N_CORES          # tokens per core = 512
TT = T // 128                   # token tiles per core = 4
CT = C // 512                   # concept tiles of 512 = 32
KC = D // 128                   # k-chunks of 128 = 8
NEG = -1.0e30

_CACHE = {}


def _build():
    nc = bacc.Bacc("TRN2", target_bir_lowering=False, debug=False,
                   num_devices=N_CORES)

    actT = nc.dram_tensor("actT", [D, T], FP32, kind="ExternalInput")
    wencT = nc.dram_tensor("wencT", [D, C], FP32, kind="ExternalInput")
    bias1 = nc.dram_tensor("bias1", [1, C], FP16, kind="ExternalInput")
    bias2 = nc.dram_tensor("bias2", [1, C], FP16, kind="ExternalInput")
    wembT = nc.dram_tensor("wembT", [C, D], FP16, kind="ExternalInput")
    out = nc.dram_tensor("out", [T, D], FP32, kind="ExternalOutput")

    with TileContext(nc) as tc:
        with (
            tc.tile_pool(name="const", bufs=1) as const_pool,
            tc.tile_pool(name="dram", bufs=1, space="DRAM") as dram_pool,
            tc.tile_pool(name="persist", bufs=1) as persist,
        ):
            ident = const_pool.tile([128, 128], FP32, tag="ident")
            make_identity(nc, ident[:])
            ones16 = const_pool.tile([1, 128], FP16, tag="ones16")
            nc.vector.memset(ones16[:], 1.0)
            ones16s = const_pool.tile([1, 128], FP16, tag="ones16s")
            nc.vector.memset(ones16s[:], 2.0 ** -8)
            ones32 = const_pool.tile([1, 128], FP32, tag="ones32")
            nc.vector.memset(ones32[:], 1.0)

            b1_all = persist.tile([1, C], FP16, tag="b1")
            nc.sync.dma_start(out=b1_all[:], in_=bias1.ap())
            b2_all = persist.tile([1, C], FP16, tag="b2")
            nc.sync.dma_start(out=b2_all[:], in_=bias2.ap())

            at = persist.tile([128, KC, T], FP32, tag="actT")
            nc.sync.dma_start(
                out=at[:], in_=actT.ap().rearrange("(o p) t -> p o t", p=128))

            cand = [persist.tile([128, 512], FP32, tag=f"cand{tt}", name=f"cand{tt}")
                    for tt in range(TT)]
            preT_scr = [dram_pool.tile([512, T], FP32, tag=f"preT{ct}", name=f"preT{ct}")
                        for ct in range(CT)]

            thr_bcast = persist.tile([128, T], FP32, tag="thr_bcast")

            # ---------------- Phase 1: encode + candidates + spill -------
            with (
                tc.tile_pool(name="wenc", bufs=2) as wenc_pool,
                tc.tile_pool(name="pre", bufs=3) as pre_pool,
                tc.tile_pool(name="trst", bufs=2) as trst_pool,
                tc.tile_pool(name="ps_enc", bufs=2, space="PSUM") as ps_enc_pool,
                tc.tile_pool(name="ps_tr", bufs=2, space="PSUM") as ps_tr_pool,
            ):
                for ct in range(CT):
                    cs = slice(ct * 512, (ct + 1) * 512)
                    wt = wenc_pool.tile([128, KC, 512], FP32, tag="wenc")
                    nc.sync.dma_start(
                        out=wt[:],
                        in_=wencT.ap()[:, cs].rearrange("(o p) n -> p o n", p=128))
                    trst = trst_pool.tile([128, 4, T], FP32, tag="trst")
                    for tt in range(TT):
                        ts = slice(tt * 128, (tt + 1) * 128)
                        ps = ps_enc_pool.tile([128, 512], FP32, tag="ps_enc")
                        for k in range(KC):
                            nc.tensor.matmul(ps[:], at[:, k, ts], wt[:, k, :],
                                             start=(k == 0), stop=False)
                        nc.tensor.matmul(ps[:], ones16[:1, :], b1_all[:1, cs],
                                         start=False, stop=False,
                                         skip_group_check=True)
                        nc.tensor.matmul(ps[:], ones16s[:1, :], b2_all[:1, cs],
                                         start=False, stop=True,
                                         skip_group_check=True)
                        pre_t = pre_pool.tile([128, 512], FP32, tag="pre")
                        nc.vector.tensor_copy(pre_t[:], ps[:])
                        # stage-1 candidates: top-8 of each 256-concept chunk
                        nc.vector.max(cand[tt][:, ct * 16: ct * 16 + 8],
                                      pre_t[:, 0:256])
                        nc.vector.max(cand[tt][:, ct * 16 + 8: ct * 16 + 16],
                                      pre_t[:, 256:512])
                        # transpose [128t, 512c] -> 4x [128c, 128t]
                        ps_tr = ps_tr_pool.tile([128, 512], FP32, tag="ps_tr")
                        for cc in range(4):
                            nc.tensor.transpose(
                                ps_tr[:, cc * 128:(cc + 1) * 128],
                                pre_t[:, cc * 128:(cc + 1) * 128], ident[:])
                        for cc in range(4):
                            nc.scalar.copy(trst[:, cc, ts],
                                           ps_tr[:, cc * 128:(cc + 1) * 128])
                    for cc in range(4):
                        nc.sync.dma_start(
                            out=preT_scr[ct][cc * 128:(cc + 1) * 128, :],
                            in_=trst[:, cc, :])

            # ---------------- Phase 1.5: top-32 threshold ----------------
            with (
                tc.tile_pool(name="small", bufs=1) as small_pool,
                tc.tile_pool(name="ps_thr", bufs=1, space="PSUM") as ps_thr_pool,
            ):
                ps_thr = ps_thr_pool.tile([1, T], FP32, tag="ps_thr")
                for tt in range(TT):
                    top32 = small_pool.tile([128, 32], FP32, tag=f"top32_{tt}")
                    for it in range(4):
                        nc.vector.max(top32[:, it * 8:(it + 1) * 8], cand[tt][:])
                        nc.vector.match_replace(
                            cand[tt][:], in_to_replace=top32[:, it * 8:(it + 1) * 8],
                            in_values=cand[tt][:], imm_value=NEG)
                    thr_col = small_pool.tile([128, 1], FP32, tag=f"thr_{tt}")
                    nc.vector.tensor_reduce(thr_col[:], top32[:],
                                            axis=mybir.AxisListType.X,
                                            op=mybir.AluOpType.min)
                    nc.tensor.transpose(ps_thr[:1, tt * 128:(tt + 1) * 128],
                                        thr_col[:], ident[:])
                thr_row = small_pool.tile([1, T], FP32, tag="thr_row")
                nc.scalar.copy(thr_row[:], ps_thr[:])
                ps_b = ps_thr_pool.tile([128, T], FP32, tag="ps_b")
                nc.tensor.matmul(ps_b[:], ones32[:1, :], thr_row[:1, :],
                                 start=True, stop=True)
                nc.vector.tensor_copy(thr_bcast[:], ps_b[:])

            # ---------------- Phase 2: mask + decode ---------------------
            with (
                tc.tile_pool(name="wemb", bufs=2) as wemb_pool,
                tc.tile_pool(name="pret", bufs=2) as pret_pool,
                tc.tile_pool(name="mask", bufs=3) as mask_pool,
                tc.tile_pool(name="ps_dec", bufs=1, space="PSUM") as ps_dec_pool,
            ):
                ps_dec = [[ps_dec_pool.tile([128, 512], FP32, tag=f"dec{m}_{n}",
                                          name=f"dec{m}_{n}")
                           for n in range(2)] for m in range(TT)]
                for ct in range(CT):
                    cs = slice(ct * 512, (ct + 1) * 512)
                    wm = wemb_pool.tile([128, 4, D], FP16, tag="wemb")
                    nc.sync.dma_start(
                        out=wm[:],
                        in_=wembT.ap()[cs, :].rearrange("(o p) n -> p o n", p=128))
                    pt = pret_pool.tile([128, 4, T], FP32, tag="pret")
                    nc.sync.dma_start(
                        out=pt[:],
                        in_=preT_scr[ct][:].rearrange("(o p) t -> p o t", p=128))
                    for cc in range(4):
                        ind = mask_pool.tile([128, T], FP32, tag="ind")
                        nc.vector.tensor_tensor(ind[:], pt[:, cc, :], thr_bcast[:],
                                                op=mybir.AluOpType.is_ge)
                        sp = mask_pool.tile([128, T], FP16, tag="sp")
                        nc.vector.tensor_tensor(sp[:], pt[:, cc, :], ind[:],
                                                op=mybir.AluOpType.mult)
                        last = (ct == CT - 1 and cc == 3)
                        for m in range(TT):
                            for n in range(2):
                                nc.tensor.matmul(
                                    ps_dec[m][n][:],
                                    sp[:, m * 128:(m + 1) * 128],
                                    wm[:, cc, n * 512:(n + 1) * 512],
                                    start=(ct == 0 and cc == 0), stop=last)
                with tc.tile_pool(name="outp", bufs=3) as out_pool:
                    for m in range(TT):
                        for n in range(2):
                            oc = out_pool.tile([128, 512], FP32, tag="oc")
                            nc.scalar.copy(oc[:], ps_dec[m][n][:])
                            nc.sync.dma_start(
                                out=out.ap()[m * 128:(m + 1) * 128,
                                             n * 512:(n + 1) * 512],
                                in_=oc[:])
    nc.compile()
    return nc


def get_nc():
    if "nc" not in _CACHE:
        _CACHE["nc"] = _build()
    return _CACHE["nc"]


def prepare_in_maps(activations, W_enc_w, W_enc_b, W_emb_w):
    """Host-side layout prep: slices + transposed contiguous views, fp16 casts."""
    act = np.ascontiguousarray(activations.reshape(B * S, D))
    wencT = np.ascontiguousarray(W_enc_w.T)          # [D, C] fp32
    b16 = W_enc_b.astype(np.float16)                 # bias high limb
    bres = (W_enc_b.astype(np.float64)
            - b16.astype(np.float64)) * 256.0        # residual * 2^8
    b2 = bres.astype(np.float16)
    wembT = np.ascontiguousarray(W_emb_w.T).astype(np.float16)  # [C, D]

    in_maps = []
    for c in range(N_CORES):
        tok = slice(c * T, (c + 1) * T)
        actT = np.ascontiguousarray(act[tok].T)      # [D, T]
        in_maps.append({
            "actT": actT,
            "wencT": wencT,
            "bias1": b16.reshape(1, C),
            "bias2": b2.reshape(1, C),
            "wembT": wembT,
        })
    return in_maps


def kernel(activations, W_enc_w, W_enc_b, W_emb_w, k):
    assert int(k) == K_TOP
    activations = np.asarray(activations, dtype=np.float32)
    W_enc_w = np.asarray(W_enc_w, dtype=np.float32)
    W_enc_b = np.asarray(W_enc_b, dtype=np.float32)
    W_emb_w = np.asarray(W_emb_w, dtype=np.float32)

    nc = get_nc()
    in_maps = prepare_in_maps(activations, W_enc_w, W_enc_b, W_emb_w)
    res = run_bass_kernel_spmd(nc, in_maps, core_ids=list(range(N_CORES)))
    out = np.concatenate([r["out"] for r in res.results], axis=0)
    return out.reshape(B, S, D)
